# revision 3
# baseline (speedup 1.0000x reference)
"""Causal multi-head attention (B=4, S=2048, D=768, H=12, Dh=64) on 8 TRN2 NeuronCores.

Sharding: B x head-group. Core c handles batch b = c//2, heads 6g..6g+5 with
g = c%2. Each core computes QKV projections for its 6 heads, causal
flash-style attention in scores-transposed layout, and a partial W_O
contraction. Host sums the two per-batch partials and adds b_O.

No collectives: per-core outputs are disjoint-summable partials.
"""
import sys

if "/opt/trn_rl_repo" not in sys.path:
    sys.path.insert(0, "/opt/trn_rl_repo")

import contextlib

import ml_dtypes
import numpy as np

import concourse.bass as bass
import concourse.tile as tile
from concourse import bacc, mybir
from concourse import bass_utils

F32 = mybir.dt.float32
F32R = mybir.dt.float32r
BF16 = mybir.dt.bfloat16
Identity = mybir.ActivationFunctionType.Identity
Exp = mybir.ActivationFunctionType.Exp

B, S, D, H, Dh = 4, 2048, 768, 12, 64
HL = 6          # heads per core
NE = HL * Dh    # 384 he-dims per core
NC_D = D // 128   # 6 d chunks
NC_E = NE // 128  # 3 he chunks
QB = 512        # q block
NQB = S // QB   # 4
NKT = S // 128  # 16 k tiles
VW = Dh + 1     # 65: v + ones column
SCALE = 1.0 / np.sqrt(Dh)

_CACHE = {}


def _build():
    nc = bacc.Bacc("TRN2", target_bir_lowering=False, debug=False, num_devices=8)
    xt_d = nc.dram_tensor("xt", [D, S], F32R, kind="ExternalInput")
    wq_d = nc.dram_tensor("wq", [D, NE], F32R, kind="ExternalInput")
    wk_d = nc.dram_tensor("wk", [D, NE], F32R, kind="ExternalInput")
    wv_d = nc.dram_tensor("wv", [D, NE], F32R, kind="ExternalInput")
    wo_d = nc.dram_tensor("wo", [NE, D], BF16, kind="ExternalInput")
    bq_d = nc.dram_tensor("bq", [128, NC_E], F32, kind="ExternalInput")
    bk_d = nc.dram_tensor("bk", [128, NC_E], F32, kind="ExternalInput")
    bv_d = nc.dram_tensor("bv", [1, HL * VW], F32, kind="ExternalInput")
    mask_d = nc.dram_tensor("mask", [128, 128], BF16, kind="ExternalInput")
    out_d = nc.dram_tensor("out", [S, D], F32, kind="ExternalOutput")

    with tile.TileContext(nc) as tc:
        with contextlib.ExitStack() as ctx:
            sb = ctx.enter_context(tc.tile_pool(name="sb", bufs=1))
            pt_pool = ctx.enter_context(tc.tile_pool(name="pt", bufs=4))
            sm_pool = ctx.enter_context(tc.tile_pool(name="sm", bufs=3))
            o_pool = ctx.enter_context(tc.tile_pool(name="o", bufs=3))
            ps_mm = ctx.enter_context(tc.tile_pool(name="psmm", bufs=3, space="PSUM"))
            ps_z = ctx.enter_context(tc.tile_pool(name="psz", bufs=2, space="PSUM"))

            # ---- persistent SBUF ----
            xt = sb.tile([128, NC_D * S], F32R, tag="xt")
            wq = sb.tile([128, NC_D * NE], F32R, tag="wq")
            wk = sb.tile([128, NC_D * NE], F32R, tag="wk")
            wv = sb.tile([128, NC_D * NE], F32R, tag="wv")
            wo = sb.tile([128, NC_E * D], BF16, tag="wo")
            bq = sb.tile([128, NC_E], F32, tag="bq")
            bk = sb.tile([128, NC_E], F32, tag="bk")
            bvr = sb.tile([1, HL * VW], F32, tag="bvr")
            bvb = sb.tile([128, HL * VW], F32, tag="bvb")
            mask = sb.tile([128, 128], BF16, tag="mask")
            ones_b = sb.tile([1, 64], BF16, tag="ones_b")
            ones_f = sb.tile([1, 128], F32, tag="ones_f")
            qt = sb.tile([128, NC_E * S], BF16, tag="qt")
            kt = sb.tile([128, NC_E * S], BF16, tag="kt")
            va = sb.tile([128, NKT * HL * VW], BF16, tag="va")
            znt = sb.tile([128, NC_E * S], BF16, tag="znt")

            # ---- input DMAs ----
            wq_r = wq_d.ap().rearrange("(c p) e -> p c e", p=128)
            wk_r = wk_d.ap().rearrange("(c p) e -> p c e", p=128)
            wv_r = wv_d.ap().rearrange("(c p) e -> p c e", p=128)
            wq_s = wq[:].rearrange("p (c e) -> p c e", c=NC_D)
            wk_s = wk[:].rearrange("p (c e) -> p c e", c=NC_D)
            wv_s = wv[:].rearrange("p (c e) -> p c e", c=NC_D)
            for c in range(2):
                h3 = 3 * c
                nc.sync.dma_start(wq_s[:, h3:h3 + 3, :], wq_r[:, h3:h3 + 3, :])
                nc.sync.dma_start(wk_s[:, h3:h3 + 3, :], wk_r[:, h3:h3 + 3, :])
                nc.sync.dma_start(wv_s[:, h3:h3 + 3, :], wv_r[:, h3:h3 + 3, :])
            nc.sync.dma_start(bq[:], bq_d.ap())
            nc.sync.dma_start(bk[:], bk_d.ap())
            nc.sync.dma_start(bvr[:], bv_d.ap())
            nc.sync.dma_start(mask[:], mask_d.ap())
            wo_r = wo_d.ap().rearrange("(c p) d -> p c d", p=128)
            wo_s = wo[:].rearrange("p (c d) -> p c d", c=NC_E)
            nc.sync.dma_start(wo_s[:], wo_r[:])
            xt_r = xt_d.ap().rearrange("(c p) s -> p c s", p=128)
            xt_s = xt[:].rearrange("p (c s) -> p c s", c=NC_D)
            for c in range(NC_D):
                for hh in range(2):
                    s0 = hh * (S // 2)
                    nc.sync.dma_start(xt_s[:, c, s0:s0 + S // 2],
                                      xt_r[:, c, s0:s0 + S // 2])

            # ---- constants ----
            nc.vector.memset(ones_b[:], 1.0)
            nc.vector.memset(ones_f[:], 1.0)
            # ones columns of V_aug
            va_4d = va[:].rearrange("p (s h e) -> p s h e", s=NKT, h=HL)
            nc.vector.memset(va_4d[:, :, :, Dh:Dh + 1], 1.0)
            # broadcast b_V row to all 128 partitions (one-time, fp32 matmul)
            bv_ps = ps_mm.tile([128, HL * VW], F32, tag="mm")
            nc.tensor.matmul(bv_ps[:], ones_f[0:1, :], bvr[:], start=True, stop=True)
            nc.scalar.copy(bvb[:], bv_ps[:])

            # ---- QKV projections ----
            for ce in range(NC_E):
                for sblk in range(NQB):
                    s0 = sblk * QB
                    psq = ps_mm.tile([128, QB], F32, tag="mm")
                    psk = ps_mm.tile([128, QB], F32, tag="mm")
                    for kc in range(NC_D):
                        lq = wq[:, kc * NE + ce * 128:kc * NE + ce * 128 + 128]
                        lk = wk[:, kc * NE + ce * 128:kc * NE + ce * 128 + 128]
                        r = xt[:, kc * S + s0:kc * S + s0 + QB]
                        nc.tensor.matmul(psq[:], lq, r, start=(kc == 0), stop=(kc == NC_D - 1))
                        nc.tensor.matmul(psk[:], lk, r, start=(kc == 0), stop=(kc == NC_D - 1))
                    nc.vector.tensor_scalar_add(qt[:, ce * S + s0:ce * S + s0 + QB],
                                                psq[:], bq[:, ce:ce + 1])
                    nc.vector.tensor_scalar_add(kt[:, ce * S + s0:ce * S + s0 + QB],
                                                psk[:], bk[:, ce:ce + 1])
            for st in range(NKT):
                psv = ps_mm.tile([128, NE], F32, tag="mm")
                for kc in range(NC_D):
                    lx = xt[:, kc * S + st * 128:kc * S + st * 128 + 128]
                    nc.tensor.matmul(psv[:], lx, wv[:, kc * NE:kc * NE + NE],
                                     start=(kc == 0), stop=(kc == NC_D - 1))
                for h in range(HL):
                    o = st * HL * VW + h * VW
                    nc.vector.tensor_add(va[:, o:o + Dh], psv[:, h * Dh:h * Dh + Dh],
                                         bvb[:, h * VW:h * VW + Dh])

            # ---- attention ----
            for h in range(HL):
                ce, sub = h // 2, h % 2
                p0 = 64 * sub
                for j in range(NQB):
                    q0 = j * QB
                    zps = ps_z.tile([128, QB], F32, tag="z")
                    nkt = 4 * (j + 1)
                    for kti in range(nkt):
                        o = kti - 4 * j  # >=0 only for diagonal tiles
                        qoff = 128 * o if o > 0 else 0
                        n = QB - qoff
                        pss = ps_mm.tile([128, QB], F32, tag="mm")
                        lhsT = kt[p0:p0 + 64, ce * S + kti * 128:ce * S + kti * 128 + 128]
                        rhs = qt[p0:p0 + 64, ce * S + q0 + qoff:ce * S + q0 + QB]
                        nc.tensor.matmul(pss[:, 0:n], lhsT, rhs, start=True, stop=True)
                        pt = pt_pool.tile([128, QB], BF16)
                        nc.scalar.activation(pt[:, 0:n], pss[:, 0:n], Exp, scale=SCALE)
                        if o >= 0:
                            nc.vector.tensor_mul(pt[:, 0:128], pt[:, 0:128], mask[:])
                        va_o = kti * HL * VW + h * VW
                        nc.tensor.matmul(zps[0:VW, qoff:QB], va[:, va_o:va_o + VW],
                                         pt[:, 0:n], start=(kti == 0), stop=(kti == nkt - 1),
                                         skip_group_check=True)
                    lr = sm_pool.tile([1, QB], BF16, tag="lr")
                    with nc.allow_low_precision(reason="softmax denom bf16"):
                        nc.vector.reciprocal(lr[:], zps[Dh:Dh + 1, :])
                    bps = ps_mm.tile([128, QB], F32, tag="mm")
                    nc.tensor.matmul(bps[0:64, :], ones_b[:], lr[:], start=True, stop=True)
                    bsb = sm_pool.tile([128, QB], BF16, tag="bsb")
                    nc.scalar.copy(bsb[0:64, :], bps[0:64, :])
                    nc.vector.tensor_mul(znt[p0:p0 + 64, ce * S + q0:ce * S + q0 + QB],
                                         zps[0:64, :], bsb[0:64, :])

            # ---- output projection (partial over this core's heads) ----
            for st in range(NKT):
                osb = o_pool.tile([128, D], F32)
                for dh in range(2):
                    pso = ps_mm.tile([128, 384], F32, tag="mm")
                    for c in range(NC_E):
                        lhsT = znt[:, c * S + st * 128:c * S + st * 128 + 128]
                        rhs = wo[:, c * D + dh * 384:c * D + dh * 384 + 384]
                        nc.tensor.matmul(pso[:], lhsT, rhs, start=(c == 0), stop=(c == NC_E - 1))
                    nc.scalar.copy(osb[:, dh * 384:dh * 384 + 384], pso[:])
                nc.sync.dma_start(out_d.ap()[st * 128:st * 128 + 128, :], osb[:])

    nc.compile()
    return nc


def _in_maps(inputs):
    residual = np.asarray(inputs["residual"], np.float32)
    W_Q = np.asarray(inputs["W_Q"], np.float32)
    W_K = np.asarray(inputs["W_K"], np.float32)
    W_V = np.asarray(inputs["W_V"], np.float32)
    W_O = np.asarray(inputs["W_O"], np.float32)
    b_Q = np.asarray(inputs["b_Q"], np.float32)
    b_K = np.asarray(inputs["b_K"], np.float32)
    b_V = np.asarray(inputs["b_V"], np.float32)
    mask = (np.arange(128)[:, None] <= np.arange(128)[None, :]).astype(ml_dtypes.bfloat16)
    maps = []
    for c in range(8):
        b, g = c // 2, c % 2
        hs = slice(HL * g, HL * g + HL)
        xt = np.ascontiguousarray(residual[b].T)
        wq = np.ascontiguousarray(np.transpose(W_Q[hs], (1, 0, 2)).reshape(D, NE))
        wk = np.ascontiguousarray(np.transpose(W_K[hs], (1, 0, 2)).reshape(D, NE))
        wv = np.ascontiguousarray(np.transpose(W_V[hs], (1, 0, 2)).reshape(D, NE))
        wo = np.ascontiguousarray(W_O[hs].reshape(NE, D)).astype(ml_dtypes.bfloat16)
        bq = np.ascontiguousarray(b_Q[hs].reshape(NC_E, 128).T)
        bk = np.ascontiguousarray(b_K[hs].reshape(NC_E, 128).T)
        bv = np.zeros((1, HL * VW), np.float32)
        for h in range(HL):
            bv[0, h * VW:h * VW + Dh] = b_V[HL * g + h]
        maps.append({"xt": xt, "wq": wq, "wk": wk, "wv": wv, "wo": wo,
                     "bq": bq, "bk": bk, "bv": bv, "mask": mask})
    return maps


def _run(inputs, trace=False, **kw):
    if "nc" not in _CACHE:
        _CACHE["nc"] = _build()
    nc = _CACHE["nc"]
    res = bass_utils.run_bass_kernel_spmd(nc, _in_maps(inputs),
                                          core_ids=list(range(8)), trace=trace, **kw)
    b_O = np.asarray(inputs["b_O"], np.float32)
    out = np.empty((B, S, D), np.float32)
    for b in range(B):
        out[b] = res.results[2 * b]["out"] + res.results[2 * b + 1]["out"] + b_O
    return out, res


def kernel(**inputs):
    out, _ = _run(inputs)
    return out


# revision 4
# speedup vs baseline: 1.4081x; 1.4081x over previous
"""Causal multi-head attention (B=4, S=2048, D=768, H=12, Dh=64) on 8 TRN2 NeuronCores.

Sharding: B x head-group. Core c handles batch b = c//2, heads 6g..6g+5 with
g = c%2. Each core computes QKV projections for its 6 heads, causal
flash-style attention in scores-transposed layout, and a partial W_O
contraction. Host sums the two per-batch partials and adds b_O.

No collectives: per-core outputs are disjoint-summable partials.
"""
import sys

if "/opt/trn_rl_repo" not in sys.path:
    sys.path.insert(0, "/opt/trn_rl_repo")

import contextlib

import ml_dtypes
import numpy as np

import concourse.bass as bass
import concourse.tile as tile
from concourse import bacc, mybir
from concourse import bass_utils

F32 = mybir.dt.float32
F32R = mybir.dt.float32r
BF16 = mybir.dt.bfloat16
Exp = mybir.ActivationFunctionType.Exp

B, S, D, H, Dh = 4, 2048, 768, 12, 64
HL = 6          # heads per core
NE = HL * Dh    # 384 he-dims per core
NC_D = D // 128   # 6 d chunks
NC_E = NE // 128  # 3 he chunks
QB = 512        # q block
NQB = S // QB   # 4
NKT = S // 128  # 16 k tiles
VW = Dh + 1     # 65: v + ones column
SCALE = 1.0 / np.sqrt(Dh)

_CACHE = {}


def _build():
    nc = bacc.Bacc("TRN2", target_bir_lowering=False, debug=False, num_devices=8)
    xt_d = nc.dram_tensor("xt", [D, S], F32R, kind="ExternalInput")
    wq_d = nc.dram_tensor("wq", [D, NE], F32R, kind="ExternalInput")
    wk_d = nc.dram_tensor("wk", [D, NE], F32R, kind="ExternalInput")
    wv_d = nc.dram_tensor("wv", [D, NE], F32R, kind="ExternalInput")
    wo_d = nc.dram_tensor("wo", [NE, D], BF16, kind="ExternalInput")
    bq_d = nc.dram_tensor("bq", [128, NC_E], F32, kind="ExternalInput")
    bk_d = nc.dram_tensor("bk", [128, NC_E], F32, kind="ExternalInput")
    bv_d = nc.dram_tensor("bv", [1, HL * VW], F32, kind="ExternalInput")
    mask_d = nc.dram_tensor("mask", [128, 128], BF16, kind="ExternalInput")
    out_d = nc.dram_tensor("out", [S, D], F32, kind="ExternalOutput")

    with tile.TileContext(nc) as tc:
        with contextlib.ExitStack() as ctx:
            sb = ctx.enter_context(tc.tile_pool(name="sb", bufs=1))
            pt_pool = ctx.enter_context(tc.tile_pool(name="pt", bufs=4))
            sm_pool = ctx.enter_context(tc.tile_pool(name="sm", bufs=3))
            o_pool = ctx.enter_context(tc.tile_pool(name="o", bufs=3))
            ps_s = ctx.enter_context(tc.tile_pool(name="pss", bufs=2, space="PSUM"))
            ps_mm = ctx.enter_context(tc.tile_pool(name="psmm", bufs=2, space="PSUM"))
            ps_z = ctx.enter_context(tc.tile_pool(name="psz", bufs=2, space="PSUM"))

            # ---- persistent SBUF ----
            xt = sb.tile([128, NC_D * S], F32R, tag="xt")
            wq = sb.tile([128, NC_D * NE], F32R, tag="wq")
            wk = sb.tile([128, NC_D * NE], F32R, tag="wk")
            wv = sb.tile([128, NC_D * NE], F32R, tag="wv")
            wo = sb.tile([128, NC_E * D], BF16, tag="wo")
            bq = sb.tile([128, NC_E], F32, tag="bq")
            bk = sb.tile([128, NC_E], F32, tag="bk")
            bvr = sb.tile([1, HL * VW], F32, tag="bvr")
            bvb = sb.tile([128, HL * VW], F32, tag="bvb")
            mask = sb.tile([128, 128], BF16, tag="mask")
            ones_b = sb.tile([1, 64], BF16, tag="ones_b")
            ones_f = sb.tile([1, 128], F32, tag="ones_f")
            qt = sb.tile([128, NC_E * S], BF16, tag="qt")
            kt = sb.tile([128, NC_E * S], BF16, tag="kt")
            va = sb.tile([128, NKT * HL * VW], BF16, tag="va")
            znt = sb.tile([128, NC_E * S], BF16, tag="znt")

            # ---- input DMAs ----
            wq_r = wq_d.ap().rearrange("(c p) e -> p c e", p=128)
            wk_r = wk_d.ap().rearrange("(c p) e -> p c e", p=128)
            wv_r = wv_d.ap().rearrange("(c p) e -> p c e", p=128)
            wq_s = wq[:].rearrange("p (c e) -> p c e", c=NC_D)
            wk_s = wk[:].rearrange("p (c e) -> p c e", c=NC_D)
            wv_s = wv[:].rearrange("p (c e) -> p c e", c=NC_D)
            for c in range(2):
                h3 = 3 * c
                nc.sync.dma_start(wq_s[:, h3:h3 + 3, :], wq_r[:, h3:h3 + 3, :])
                nc.sync.dma_start(wk_s[:, h3:h3 + 3, :], wk_r[:, h3:h3 + 3, :])
                nc.sync.dma_start(wv_s[:, h3:h3 + 3, :], wv_r[:, h3:h3 + 3, :])
            nc.sync.dma_start(bq[:], bq_d.ap())
            nc.sync.dma_start(bk[:], bk_d.ap())
            nc.sync.dma_start(bvr[:], bv_d.ap())
            nc.sync.dma_start(mask[:], mask_d.ap())
            wo_r = wo_d.ap().rearrange("(c p) d -> p c d", p=128)
            wo_s = wo[:].rearrange("p (c d) -> p c d", c=NC_E)
            nc.sync.dma_start(wo_s[:], wo_r[:])
            xt_r = xt_d.ap().rearrange("(c p) s -> p c s", p=128)
            xt_s = xt[:].rearrange("p (c s) -> p c s", c=NC_D)
            for c in range(NC_D):
                for hh in range(2):
                    s0 = hh * (S // 2)
                    nc.sync.dma_start(xt_s[:, c, s0:s0 + S // 2],
                                      xt_r[:, c, s0:s0 + S // 2])

            # ---- constants ----
            nc.vector.memset(ones_b[:], 1.0)
            nc.vector.memset(ones_f[:], 1.0)
            va_4d = va[:].rearrange("p (s h e) -> p s h e", s=NKT, h=HL)
            nc.vector.memset(va_4d[:, :, :, Dh:Dh + 1], 1.0)
            # broadcast b_V row to all 128 partitions (one-time, fp32 matmul)
            bv_ps = ps_mm.tile([128, HL * VW], F32, tag="mm")
            nc.tensor.matmul(bv_ps[:], ones_f[0:1, :], bvr[:], start=True, stop=True)
            nc.scalar.copy(bvb[:], bv_ps[:])

            # ---- QKV projections ----
            for ce in range(NC_E):
                for sblk in range(NQB):
                    s0 = sblk * QB
                    psq = ps_s.tile([128, QB], F32, tag="s")
                    psk = ps_mm.tile([128, QB], F32, tag="mm")
                    for kc in range(NC_D):
                        lq = wq[:, kc * NE + ce * 128:kc * NE + ce * 128 + 128]
                        lk = wk[:, kc * NE + ce * 128:kc * NE + ce * 128 + 128]
                        r = xt[:, kc * S + s0:kc * S + s0 + QB]
                        nc.tensor.matmul(psq[:, 0:QB], lq, r, start=(kc == 0), stop=(kc == NC_D - 1))
                        nc.tensor.matmul(psk[:], lk, r, start=(kc == 0), stop=(kc == NC_D - 1))
                    nc.vector.tensor_scalar_add(qt[:, ce * S + s0:ce * S + s0 + QB],
                                                psq[:, 0:QB], bq[:, ce:ce + 1])
                    nc.vector.tensor_scalar_add(kt[:, ce * S + s0:ce * S + s0 + QB],
                                                psk[:], bk[:, ce:ce + 1])
            for st in range(NKT):
                psv = ps_mm.tile([128, NE], F32, tag="mm")
                for kc in range(NC_D):
                    lx = xt[:, kc * S + st * 128:kc * S + st * 128 + 128]
                    nc.tensor.matmul(psv[:], lx, wv[:, kc * NE:kc * NE + NE],
                                     start=(kc == 0), stop=(kc == NC_D - 1))
                for h in range(HL):
                    o = st * HL * VW + h * VW
                    nc.vector.tensor_add(va[:, o:o + Dh], psv[:, h * Dh:h * Dh + Dh],
                                         bvb[:, h * VW:h * VW + Dh])

            # ---- attention ----
            def epilogue(h, j, zps):
                ce, sub = h // 2, h % 2
                p0 = 64 * sub
                q0 = j * QB
                lsb = sm_pool.tile([1, QB], BF16, tag="lsb")
                with nc.allow_low_precision(reason="softmax denom bf16"):
                    nc.vector.tensor_copy(lsb[:], zps[Dh:Dh + 1, :])
                bps = ps_mm.tile([128, QB], F32, tag="mm")
                nc.tensor.matmul(bps[0:64, :], ones_b[:], lsb[:], start=True, stop=True)
                rsb = sm_pool.tile([64, QB], F32, tag="rsb")
                nc.vector.reciprocal_approx_fast(rsb[:], bps[0:64, :])
                nc.vector.tensor_mul(znt[p0:p0 + 64, ce * S + q0:ce * S + q0 + QB],
                                     zps[0:64, :], rsb[:])

            pending = None
            for h in range(HL):
                ce, sub = h // 2, h % 2
                p0 = 64 * sub
                for j in range(NQB):
                    q0 = j * QB
                    zps = ps_z.tile([128, QB], F32, tag="z")
                    nkt = 4 * (j + 1)
                    nz = 0
                    for m in range(nkt // 2):
                        kts = (2 * m, 2 * m + 1)
                        qoffs = [128 * (k - 4 * j) if k >= 4 * j else 0 for k in kts]
                        ns = [QB - qo for qo in qoffs]
                        seg = [0, ns[0]]
                        pss = ps_s.tile([128, 2 * QB], F32, tag="s")
                        for i, k in enumerate(kts):
                            lhsT = kt[p0:p0 + 64, ce * S + k * 128:ce * S + k * 128 + 128]
                            rhs = qt[p0:p0 + 64, ce * S + q0 + qoffs[i]:ce * S + q0 + QB]
                            nc.tensor.matmul(pss[:, seg[i]:seg[i] + ns[i]], lhsT, rhs,
                                             start=True, stop=True, skip_group_check=True)
                        pt = pt_pool.tile([128, 2 * QB], BF16)
                        nc.scalar.activation(pt[:, 0:ns[0] + ns[1]], pss[:, 0:ns[0] + ns[1]],
                                             Exp, scale=SCALE)
                        for i, k in enumerate(kts):
                            if k >= 4 * j:
                                nc.vector.tensor_mul(pt[:, seg[i]:seg[i] + 128],
                                                     pt[:, seg[i]:seg[i] + 128], mask[:])
                        for i, k in enumerate(kts):
                            va_o = k * HL * VW + h * VW
                            nc.tensor.matmul(zps[0:VW, qoffs[i]:QB], va[:, va_o:va_o + VW],
                                             pt[:, seg[i]:seg[i] + ns[i]],
                                             start=(nz == 0), stop=(nz == nkt - 1),
                                             skip_group_check=True)
                            nz += 1
                    if pending is not None:
                        epilogue(*pending)
                    pending = (h, j, zps)
            epilogue(*pending)

            # ---- output projection (partial over this core's heads) ----
            for st in range(NKT):
                osb = o_pool.tile([128, D], F32)
                for dh in range(2):
                    pso = ps_mm.tile([128, 384], F32, tag="mm")
                    for c in range(NC_E):
                        lhsT = znt[:, c * S + st * 128:c * S + st * 128 + 128]
                        rhs = wo[:, c * D + dh * 384:c * D + dh * 384 + 384]
                        nc.tensor.matmul(pso[:], lhsT, rhs, start=(c == 0), stop=(c == NC_E - 1))
                    nc.scalar.copy(osb[:, dh * 384:dh * 384 + 384], pso[:])
                nc.sync.dma_start(out_d.ap()[st * 128:st * 128 + 128, :], osb[:])

    nc.compile()
    return nc


def _in_maps(inputs):
    residual = np.asarray(inputs["residual"], np.float32)
    W_Q = np.asarray(inputs["W_Q"], np.float32)
    W_K = np.asarray(inputs["W_K"], np.float32)
    W_V = np.asarray(inputs["W_V"], np.float32)
    W_O = np.asarray(inputs["W_O"], np.float32)
    b_Q = np.asarray(inputs["b_Q"], np.float32)
    b_K = np.asarray(inputs["b_K"], np.float32)
    b_V = np.asarray(inputs["b_V"], np.float32)
    mask = (np.arange(128)[:, None] <= np.arange(128)[None, :]).astype(ml_dtypes.bfloat16)
    maps = []
    for c in range(8):
        b, g = c // 2, c % 2
        hs = slice(HL * g, HL * g + HL)
        xt = np.ascontiguousarray(residual[b].T)
        wq = np.ascontiguousarray(np.transpose(W_Q[hs], (1, 0, 2)).reshape(D, NE))
        wk = np.ascontiguousarray(np.transpose(W_K[hs], (1, 0, 2)).reshape(D, NE))
        wv = np.ascontiguousarray(np.transpose(W_V[hs], (1, 0, 2)).reshape(D, NE))
        wo = np.ascontiguousarray(W_O[hs].reshape(NE, D)).astype(ml_dtypes.bfloat16)
        bq = np.ascontiguousarray(b_Q[hs].reshape(NC_E, 128).T)
        bk = np.ascontiguousarray(b_K[hs].reshape(NC_E, 128).T)
        bv = np.zeros((1, HL * VW), np.float32)
        for h in range(HL):
            bv[0, h * VW:h * VW + Dh] = b_V[HL * g + h]
        maps.append({"xt": xt, "wq": wq, "wk": wk, "wv": wv, "wo": wo,
                     "bq": bq, "bk": bk, "bv": bv, "mask": mask})
    return maps


def _run(inputs, trace=False, **kw):
    if "nc" not in _CACHE:
        _CACHE["nc"] = _build()
    nc = _CACHE["nc"]
    res = bass_utils.run_bass_kernel_spmd(nc, _in_maps(inputs),
                                          core_ids=list(range(8)), trace=trace, **kw)
    b_O = np.asarray(inputs["b_O"], np.float32)
    out = np.empty((B, S, D), np.float32)
    for b in range(B):
        out[b] = res.results[2 * b]["out"] + res.results[2 * b + 1]["out"] + b_O
    return out, res


def kernel(**inputs):
    out, _ = _run(inputs)
    return out


# revision 7
# speedup vs baseline: 1.6285x; 1.1565x over previous
"""Causal multi-head attention (B=4, S=2048, D=768, H=12, Dh=64) on 8 TRN2 NeuronCores.

Sharding: B x head-group. Core c handles batch b = c//2, heads 6g..6g+5 with
g = c%2. Each core computes QKV projections for its 6 heads, causal
flash-style attention in scores-transposed layout, and a partial W_O
contraction. Host sums the two per-batch partials and adds b_O.

Structure: j-major over q-blocks; projection / W_O matmuls are interleaved as
fillers between attention tile-pairs so TensorE stays dense (keeps the HAM
clock gate at full rate) while ScalarE runs the exp stream.

No collectives: per-core outputs are disjoint-summable partials.
"""
import sys

if "/opt/trn_rl_repo" not in sys.path:
    sys.path.insert(0, "/opt/trn_rl_repo")

import contextlib

import ml_dtypes
import numpy as np

import concourse.bass as bass
import concourse.tile as tile
from concourse import bacc, mybir
from concourse import bass_utils

F32 = mybir.dt.float32
F32R = mybir.dt.float32r
BF16 = mybir.dt.bfloat16
Exp = mybir.ActivationFunctionType.Exp

B, S, D, H, Dh = 4, 2048, 768, 12, 64
HL = 6          # heads per core
NE = HL * Dh    # 384 he-dims per core
NC_D = D // 128   # 6 d chunks
NC_E = NE // 128  # 3 he chunks
QB = 512        # q block
NQB = S // QB   # 4
NKT = S // 128  # 16 k tiles
VW = Dh + 1     # 65: v + ones column
SCALE = 1.0 / np.sqrt(Dh)

_CACHE = {}


def _build():
    nc = bacc.Bacc("TRN2", target_bir_lowering=False, debug=False, num_devices=8)
    xt_d = nc.dram_tensor("xt", [D, S], F32R, kind="ExternalInput")
    wq_d = nc.dram_tensor("wq", [D, NE], F32R, kind="ExternalInput")
    wk_d = nc.dram_tensor("wk", [D, NE], F32R, kind="ExternalInput")
    wv_d = nc.dram_tensor("wv", [D, NE], F32R, kind="ExternalInput")
    wo_d = nc.dram_tensor("wo", [NE, D], BF16, kind="ExternalInput")
    bq_d = nc.dram_tensor("bq", [128, NC_E], F32, kind="ExternalInput")
    bk_d = nc.dram_tensor("bk", [128, NC_E], F32, kind="ExternalInput")
    bv_d = nc.dram_tensor("bv", [1, HL * VW], F32, kind="ExternalInput")
    mask_d = nc.dram_tensor("mask", [128, 128], BF16, kind="ExternalInput")
    out_d = nc.dram_tensor("out", [S, D], F32, kind="ExternalOutput")

    with tile.TileContext(nc) as tc:
        with contextlib.ExitStack() as ctx:
            sb = ctx.enter_context(tc.tile_pool(name="sb", bufs=1))
            pt_pool = ctx.enter_context(tc.tile_pool(name="pt", bufs=4))
            sm_pool = ctx.enter_context(tc.tile_pool(name="sm", bufs=3))
            o_pool = ctx.enter_context(tc.tile_pool(name="o", bufs=3))
            ps_s = ctx.enter_context(tc.tile_pool(name="pss", bufs=2, space="PSUM"))
            ps_mm = ctx.enter_context(tc.tile_pool(name="psmm", bufs=2, space="PSUM"))
            ps_z = ctx.enter_context(tc.tile_pool(name="psz", bufs=2, space="PSUM"))

            # ---- persistent SBUF ----
            xt = sb.tile([128, NC_D * S], F32R, tag="xt")
            wq = sb.tile([128, NC_D * NE], F32R, tag="wq")
            wk = sb.tile([128, NC_D * NE], F32R, tag="wk")
            wv = sb.tile([128, NC_D * NE], F32R, tag="wv")
            wo = sb.tile([128, NC_E * D], BF16, tag="wo")
            bq = sb.tile([128, NC_E], F32, tag="bq")
            bk = sb.tile([128, NC_E], F32, tag="bk")
            bvr = sb.tile([1, HL * VW], F32, tag="bvr")
            bvb = sb.tile([128, HL * VW], F32, tag="bvb")
            mask = sb.tile([128, 128], BF16, tag="mask")
            ones_b = sb.tile([1, 64], BF16, tag="ones_b")
            ones_f = sb.tile([1, 128], F32, tag="ones_f")
            qt = sb.tile([128, NC_E * S], BF16, tag="qt")
            kt = sb.tile([128, NC_E * S], BF16, tag="kt")
            va = sb.tile([128, NKT * HL * VW], BF16, tag="va")
            znt = sb.tile([128, NC_E * S], BF16, tag="znt")

            # ---- input DMAs ----
            wq_r = wq_d.ap().rearrange("(c p) e -> p c e", p=128)
            wk_r = wk_d.ap().rearrange("(c p) e -> p c e", p=128)
            wv_r = wv_d.ap().rearrange("(c p) e -> p c e", p=128)
            wq_s = wq[:].rearrange("p (c e) -> p c e", c=NC_D)
            wk_s = wk[:].rearrange("p (c e) -> p c e", c=NC_D)
            wv_s = wv[:].rearrange("p (c e) -> p c e", c=NC_D)
            for c in range(2):
                h3 = 3 * c
                nc.sync.dma_start(wq_s[:, h3:h3 + 3, :], wq_r[:, h3:h3 + 3, :])
                nc.sync.dma_start(wk_s[:, h3:h3 + 3, :], wk_r[:, h3:h3 + 3, :])
                nc.sync.dma_start(wv_s[:, h3:h3 + 3, :], wv_r[:, h3:h3 + 3, :])
            nc.sync.dma_start(bq[:], bq_d.ap())
            nc.sync.dma_start(bk[:], bk_d.ap())
            nc.sync.dma_start(bvr[:], bv_d.ap())
            nc.sync.dma_start(mask[:], mask_d.ap())
            wo_r = wo_d.ap().rearrange("(c p) d -> p c d", p=128)
            wo_s = wo[:].rearrange("p (c d) -> p c d", c=NC_E)
            nc.sync.dma_start(wo_s[:], wo_r[:])
            xt_r = xt_d.ap().rearrange("(c p) s -> p c s", p=128)
            xt_s = xt[:].rearrange("p (c s) -> p c s", c=NC_D)
            for c in range(NC_D):
                for q in range(4):
                    s0 = q * (S // 4)
                    nc.sync.dma_start(xt_s[:, c, s0:s0 + S // 4],
                                      xt_r[:, c, s0:s0 + S // 4])

            # ---- constants ----
            nc.vector.memset(ones_b[:], 1.0)
            nc.vector.memset(ones_f[:], 1.0)
            va_4d = va[:].rearrange("p (s h e) -> p s h e", s=NKT, h=HL)
            nc.vector.memset(va_4d[:, :, :, Dh:Dh + 1], 1.0)
            bv_ps = ps_mm.tile([128, HL * VW], F32, tag="mm")
            nc.tensor.matmul(bv_ps[:], ones_f[0:1, :], bvr[:], start=True, stop=True)
            nc.scalar.copy(bvb[:], bv_ps[:])

            # ---- work units (each emits one PE matmul + epilogue ops) ----
            def qk_proj_units(sblk):
                s0 = sblk * QB
                for ce in range(NC_E):
                    state = {}

                    def unit(ce=ce, state=state):
                        kc = state.setdefault("kc", 0)
                        if kc == 0:
                            state["psq"] = ps_mm.tile([128, QB], F32, tag="mm", name="psq")
                            state["psk"] = ps_mm.tile([128, QB], F32, tag="mm", name="psk")
                        lq = wq[:, kc * NE + ce * 128:kc * NE + ce * 128 + 128]
                        lk = wk[:, kc * NE + ce * 128:kc * NE + ce * 128 + 128]
                        r = xt[:, kc * S + s0:kc * S + s0 + QB]
                        st_, sp = (kc == 0), (kc == NC_D - 1)
                        nc.tensor.matmul(state["psq"][:], lq, r, start=st_, stop=sp)
                        nc.tensor.matmul(state["psk"][:], lk, r, start=st_, stop=sp)
                        if sp:
                            nc.vector.tensor_scalar_add(
                                qt[:, ce * S + s0:ce * S + s0 + QB], state["psq"][:],
                                bq[:, ce:ce + 1])
                            nc.vector.tensor_scalar_add(
                                kt[:, ce * S + s0:ce * S + s0 + QB], state["psk"][:],
                                bk[:, ce:ce + 1])
                        state["kc"] = kc + 1

                    for _ in range(NC_D):
                        yield unit

            def v_proj_units(st):
                state = {}

                def unit(state=state):
                    kc = state.setdefault("kc", 0)
                    if kc == 0:
                        state["psv"] = ps_mm.tile([128, NE], F32, tag="mm", name="psv")
                    lx = xt[:, kc * S + st * 128:kc * S + st * 128 + 128]
                    nc.tensor.matmul(state["psv"][:], lx, wv[:, kc * NE:kc * NE + NE],
                                     start=(kc == 0), stop=(kc == NC_D - 1))
                    if kc == NC_D - 1:
                        for h in range(HL):
                            o = st * HL * VW + h * VW
                            nc.vector.tensor_add(va[:, o:o + Dh],
                                                 state["psv"][:, h * Dh:h * Dh + Dh],
                                                 bvb[:, h * VW:h * VW + Dh])
                    state["kc"] = kc + 1

                for _ in range(NC_D):
                    yield unit

            def wo_units(st):
                osb = [None]

                def mkunit(dh, c, osb=osb):
                    def unit():
                        if dh == 0 and c == 0:
                            osb[0] = o_pool.tile([128, D], F32, tag="osb", name="osb")
                        if c == 0:
                            osb.append(ps_mm.tile([128, 384], F32, tag="mm", name="pso"))
                        pso = osb[-1]
                        lhsT = znt[:, c * S + st * 128:c * S + st * 128 + 128]
                        rhs = wo[:, c * D + dh * 384:c * D + dh * 384 + 384]
                        nc.tensor.matmul(pso[:], lhsT, rhs, start=(c == 0),
                                         stop=(c == NC_E - 1))
                        if c == NC_E - 1:
                            nc.scalar.copy(osb[0][:, dh * 384:dh * 384 + 384], pso[:])
                            if dh == 1:
                                nc.sync.dma_start(
                                    out_d.ap()[st * 128:st * 128 + 128, :], osb[0][:])
                    return unit

                for dh in range(2):
                    for c in range(NC_E):
                        yield mkunit(dh, c)

            def rr(*gens):
                gens = [iter(g) for g in gens]
                out = []
                while gens:
                    nxt = []
                    for g in gens:
                        try:
                            out.append(next(g))
                            nxt.append(g)
                        except StopIteration:
                            pass
                    gens = nxt
                return out

            # ---- epilogue: normalize z^T by the softmax denominators ----
            def epilogue(h, j, zps):
                ce, sub = h // 2, h % 2
                p0 = 64 * sub
                q0 = j * QB
                lsb = sm_pool.tile([1, QB], BF16, tag="lsb")
                with nc.allow_low_precision(reason="softmax denom bf16"):
                    nc.vector.tensor_copy(lsb[:], zps[Dh:Dh + 1, :])
                bps = ps_s.tile([128, 2 * QB], F32, tag="s")
                nc.tensor.matmul(bps[0:64, 0:QB], ones_b[:], lsb[:], start=True, stop=True)
                rsb = sm_pool.tile([64, QB], F32, tag="rsb")
                nc.vector.reciprocal_approx_fast(rsb[:], bps[0:64, 0:QB])
                nc.vector.tensor_mul(znt[p0:p0 + 64, ce * S + q0:ce * S + q0 + QB],
                                     zps[0:64, :], rsb[:])

            # ---- prologue: projections needed before attention j=0 ----
            for u in rr(qk_proj_units(0), v_proj_units(0), v_proj_units(1),
                        v_proj_units(2), v_proj_units(3)):
                u()

            # ---- main j-major loop with filler interleave ----
            pending = None
            for j in range(NQB):
                fillers = []
                if j + 1 < NQB:
                    fillers = rr(qk_proj_units(j + 1),
                                 v_proj_units(4 * (j + 1)), v_proj_units(4 * (j + 1) + 1),
                                 v_proj_units(4 * (j + 1) + 2), v_proj_units(4 * (j + 1) + 3))
                wo_fill = []
                if j >= 1:
                    wo_fill = rr(wo_units(4 * (j - 1)), wo_units(4 * (j - 1) + 1),
                                 wo_units(4 * (j - 1) + 2), wo_units(4 * (j - 1) + 3))
                fq = list(fillers)
                wq_ = list(wo_fill)

                npairs = HL * 2 * (j + 1)
                pair_idx = 0
                for h in range(HL):
                    ce, sub = h // 2, h % 2
                    p0 = 64 * sub
                    q0 = j * QB
                    zps = ps_z.tile([128, QB], F32, tag="z")
                    nkt = 4 * (j + 1)
                    nz = 0
                    for m in range(nkt // 2):
                        kts = (2 * m, 2 * m + 1)
                        qoffs = [128 * (k - 4 * j) if k >= 4 * j else 0 for k in kts]
                        ns = [QB - qo for qo in qoffs]
                        seg = [0, ns[0]]
                        pss = ps_s.tile([128, 2 * QB], F32, tag="s")
                        for i, k in enumerate(kts):
                            lhsT = kt[p0:p0 + 64, ce * S + k * 128:ce * S + k * 128 + 128]
                            rhs = qt[p0:p0 + 64, ce * S + q0 + qoffs[i]:ce * S + q0 + QB]
                            nc.tensor.matmul(pss[:, seg[i]:seg[i] + ns[i]], lhsT, rhs,
                                             start=True, stop=True, skip_group_check=True)
                        pt = pt_pool.tile([128, 2 * QB], BF16)
                        nc.scalar.activation(pt[:, 0:ns[0] + ns[1]], pss[:, 0:ns[0] + ns[1]],
                                             Exp, scale=SCALE)
                        for i, k in enumerate(kts):
                            if k >= 4 * j:
                                nc.vector.tensor_mul(pt[:, seg[i]:seg[i] + 128],
                                                     pt[:, seg[i]:seg[i] + 128], mask[:])
                        for i, k in enumerate(kts):
                            va_o = k * HL * VW + h * VW
                            nc.tensor.matmul(zps[0:VW, qoffs[i]:QB], va[:, va_o:va_o + VW],
                                             pt[:, seg[i]:seg[i] + ns[i]],
                                             start=(nz == 0), stop=(nz == nkt - 1),
                                             skip_group_check=True)
                            nz += 1
                        # interleave filler matmuls to keep PE dense
                        pair_idx += 1
                        rem_pairs = npairs - pair_idx
                        avail = len(fq) + (len(wq_) if h >= 1 else 0)
                        take = -(-avail // max(rem_pairs, 1)) if avail else 0
                        for _ in range(take):
                            if fq:
                                fq.pop(0)()
                            elif h >= 1 and wq_:
                                wq_.pop(0)()
                    if pending is not None:
                        epilogue(*pending)
                    pending = (h, j, zps)
                # flush any leftover fillers for this j
                for u in fq:
                    u()
                for u in wq_:
                    u()
            epilogue(*pending)
            for st2 in range(4 * (NQB - 1), NKT):
                for u in wo_units(st2):
                    u()

    nc.compile()
    return nc


def _in_maps(inputs):
    residual = np.asarray(inputs["residual"], np.float32)
    W_Q = np.asarray(inputs["W_Q"], np.float32)
    W_K = np.asarray(inputs["W_K"], np.float32)
    W_V = np.asarray(inputs["W_V"], np.float32)
    W_O = np.asarray(inputs["W_O"], np.float32)
    b_Q = np.asarray(inputs["b_Q"], np.float32)
    b_K = np.asarray(inputs["b_K"], np.float32)
    b_V = np.asarray(inputs["b_V"], np.float32)
    mask = (np.arange(128)[:, None] <= np.arange(128)[None, :]).astype(ml_dtypes.bfloat16)
    maps = []
    for c in range(8):
        b, g = c // 2, c % 2
        hs = slice(HL * g, HL * g + HL)
        xt = np.ascontiguousarray(residual[b].T)
        wqm = np.ascontiguousarray(np.transpose(W_Q[hs], (1, 0, 2)).reshape(D, NE))
        wkm = np.ascontiguousarray(np.transpose(W_K[hs], (1, 0, 2)).reshape(D, NE))
        wvm = np.ascontiguousarray(np.transpose(W_V[hs], (1, 0, 2)).reshape(D, NE))
        wom = np.ascontiguousarray(W_O[hs].reshape(NE, D)).astype(ml_dtypes.bfloat16)
        bqm = np.ascontiguousarray(b_Q[hs].reshape(NC_E, 128).T)
        bkm = np.ascontiguousarray(b_K[hs].reshape(NC_E, 128).T)
        bvm = np.zeros((1, HL * VW), np.float32)
        for h in range(HL):
            bvm[0, h * VW:h * VW + Dh] = b_V[HL * g + h]
        maps.append({"xt": xt, "wq": wqm, "wk": wkm, "wv": wvm, "wo": wom,
                     "bq": bqm, "bk": bkm, "bv": bvm, "mask": mask})
    return maps


def _run(inputs, trace=False, **kw):
    if "nc" not in _CACHE:
        _CACHE["nc"] = _build()
    nc = _CACHE["nc"]
    res = bass_utils.run_bass_kernel_spmd(nc, _in_maps(inputs),
                                          core_ids=list(range(8)), trace=trace, **kw)
    b_O = np.asarray(inputs["b_O"], np.float32)
    out = np.empty((B, S, D), np.float32)
    for b in range(B):
        out[b] = res.results[2 * b]["out"] + res.results[2 * b + 1]["out"] + b_O
    return out, res


def kernel(**inputs):
    out, _ = _run(inputs)
    return out
